# revision 12
# baseline (speedup 1.0000x reference)
"""BitLinear (packed +/-1 linear layer) Trainium2 kernel.

Math: out[b,o] = sum_k a[b,k]*w[o,k] + bias[o], where a/w are +/-1 values
bit-packed LSB-first into bytes (stored as int32 0..255).

Device strategy (8 NeuronCores, data-parallel over batch):
  - Each core gets B/8 = 1024 batch rows; the full weight matrix is
    replicated. Host sends *transposed packed* uint8 tensors (k on
    partitions); the device unpacks bits into [K, *] fp8 operands.
  - Unpack: one DVE tensor_scalar per (kp-tile, bit) moves bit i of every
    byte to bit position 6 and masks: y = (x << (6-i)) & 0x40 (i=7 uses
    >> 1). Byte 0x40 bitcast to fp8e4 reads as exactly 2.0. Ops run on
    uint16 views (DVE 4x perf mode, ~355ns per [128,2048]u8 op measured).
  - fp8e4 DoubleRow matmuls (256-deep contraction), activation tile
    stationary reused across 4 PSUM banks (~120ns/matmul marginal at the
    2.4GHz max p-state; 1024 matmuls/core -> ~123us PE floor).
  - Weights stream in NH=2 phases of [K, 2048]; the w2 pool is
    double-buffered so phase h+1's DMA+unpack hides under phase h's
    matmuls (the old single-buffered pool serialized here).
  - DMA: the old kernel was DMA-bound: ~2.7us fixed cost per DMA
    instruction and a single issue queue put ~20MB of traffic at
    ~125GB/s. Now: fp16 output (8MB), batched per-kp-tile input DMAs,
    and issue spread across SP (weights), Activation (activations +
    even-b outputs), Pool (epilogue constants + odd-b outputs).
  - Identity: with a = 2*alpha-1, w = 2*omega-1 (alpha,omega in {0,1}):
      out = 4*M - 2*rowsum(alpha) - 2*rowsum(omega) + K + bias
          = psum + r2[b] + c[o]
    where r2[b] = -2*popcount_rows(A), c[o] = bias + K - 2*popcount_rows(W)
    are host precomputes. Epilogue: one scalar_tensor_tensor per psum
    bank into an fp16 staging tile, one output DMA per (b, phase).

Numerics: products in {0,4}, fp32 PSUM accumulation of integers <= 2^14 is
exact; fp32->fp16 output rounding gives |err| <= 1.0 at |out| <= ~2048
(rel ~8e-4 vs the 2e-2 gate).
"""

import os
import sys

import numpy as np

for _p in ("/opt/trn_rl_repo", "/root/.axon_site/_ro/trn_rl_repo"):
    if os.path.isdir(_p) and _p not in sys.path:
        sys.path.append(_p)

BATCH = 8192
IN_FEATURES = 4096
OUT_FEATURES = 4096
PACKED_LEN = IN_FEATURES // 8  # 512
N_CORES = 8
P = 128

_NC_CACHE: dict = {}
LAST_RESULTS = None  # stash of the most recent BassKernelResults (for test.py)


def build_program(B, O, K, n_devices=N_CORES, o_half=2048, reps=1,
                  mm_reps=1, out_bufs=3, stage_bufs=3, psum_bufs=2,
                  w2_bufs=2, out_f32=False, skip_out_dma=False,
                  skip_epilogue=False, skip_unpack=False, psum_merge=True):
    """Emit the per-core Bass/Tile program. SPMD: same program every core.

    reps>1 repeats the whole compute body (identical writes) so test.py can
    measure pure device time as (T(reps=R) - T(1)) / (R - 1). mm_reps
    repeats only the matmul block (restarting PSUM accumulation, last rep
    wins) for engine-rate microbenchmarks."""
    import concourse.bass as bass  # noqa: F401
    import concourse.mybir as mybir
    import concourse.tile as tile
    from concourse import bacc

    KP = K // 8  # packed k rows
    NT = KP // P  # kp tiles (4)
    NK2 = K // 256  # DoubleRow k-pair tiles (16)
    OH = min(O, o_half)  # o columns processed per outer phase
    NH = O // OH
    NOQ = OH // 512  # psum banks per phase
    NB = B // P  # batch tiles
    assert KP % P == 0 and O % OH == 0 and OH % 512 == 0 and B % P == 0
    assert NK2 * 2 == NT * 8 and NOQ * psum_bufs <= 8

    u8 = mybir.dt.uint8
    u16 = mybir.dt.uint16
    f32 = mybir.dt.float32
    f16 = mybir.dt.float16
    fp8 = mybir.dt.float8e4
    shl = mybir.AluOpType.logical_shift_left
    shr = mybir.AluOpType.logical_shift_right
    band = mybir.AluOpType.bitwise_and
    add = mybir.AluOpType.add
    out_dt = f32 if out_f32 else f16

    nc = bacc.Bacc(
        "TRN2",
        target_bir_lowering=False,
        debug=False,
        num_devices=n_devices,
    )

    at_d = nc.dram_tensor("at", [KP, B], u8, kind="ExternalInput").ap()
    wt_d = nc.dram_tensor("wt", [KP, O], u8, kind="ExternalInput").ap()
    c_d = nc.dram_tensor("c_rep", [P, O], f32, kind="ExternalInput").ap()
    r2_d = nc.dram_tensor("r2t", [P, NB], f32, kind="ExternalInput").ap()
    out_d = nc.dram_tensor("out", [B, O], out_dt, kind="ExternalOutput").ap()

    def unpack_ops(i):
        # Packed byte -> {0x00, 0x40} per byte lane for bit i: move the bit
        # to position 6 and mask (0x40 bitcast to fp8e4 reads as exactly
        # 2.0). Ops run on uint16 views (2 byte-lanes per element): for
        # shifts <= 6 left / 1 right, each kept bit (6 and 14) sources from
        # its own byte, so lanes stay independent under the 0x4040 mask.
        # bitVec ops keep in/out dtype equal (walrus rule) and the 16-bit
        # dtype enables the DVE 4x perf mode.
        return (shr, 1, band, 0x4040) if i == 7 else (shl, 6 - i, band, 0x4040)

    with tile.TileContext(nc) as tc:
        with (
            tc.tile_pool(name="consts", bufs=1) as cpool,
            tc.tile_pool(name="a2", bufs=1) as a2pool,
            tc.tile_pool(name="w2", bufs=w2_bufs) as w2pool,
            tc.tile_pool(name="stage", bufs=stage_bufs) as spool,
            tc.tile_pool(name="astage", bufs=2) as aspool,
            tc.tile_pool(name="outs", bufs=out_bufs) as opool,
            tc.tile_pool(name="psum", bufs=psum_bufs, space="PSUM") as ppool,
        ):
            c_rep = cpool.tile([P, O], f32, name="c_rep_t")
            r2t = cpool.tile([P, NB], f32, name="r2t_t")
            consts_loaded = False

            # repeat body for delta timing (rep>0 re-does identical work)
            for rep in range(reps):
              # ---- unpack activations (whole batch shard, kept resident) ----
              a2 = [
                  a2pool.tile([P, 2, B], u8, name=f"a2_{k2}")
                  for k2 in range(NK2)
              ]
              for h in range(NH):
                  # ---- unpack this phase's weight slice; in the first phase
                  # the activation unpack is interleaved (k2-major) so the PE
                  # can start as soon as the first a2/w2 pair lands ----
                  w2 = [
                      w2pool.tile([P, 2, OH], u8, name=f"w2_{k2}")
                      for k2 in range(NK2)
                  ]
                  for t in range(NT):
                      wt_st = spool.tile([P, OH], u8, name="wt_st")
                      nc.sync.dma_start(
                          out=wt_st,
                          in_=wt_d[t * P : (t + 1) * P, h * OH : (h + 1) * OH],
                      )
                      if h == 0:
                          at_st = aspool.tile([P, B], u8, name="at_st")
                          nc.scalar.dma_start(
                              out=at_st, in_=at_d[t * P : (t + 1) * P, :]
                          )
                      if not consts_loaded:
                          # on the idle Pool queue: lands well before the
                          # first epilogue without delaying critical tiles
                          consts_loaded = True
                          nc.gpsimd.dma_start(out=r2t, in_=r2_d)
                          nc.gpsimd.dma_start(out=c_rep, in_=c_d)
                      for i in range(8):
                          op0, s1, op1, s2 = unpack_ops(i)
                          # skip_unpack: timing-only variant — replace the
                          # full-width unpack with a 16-element write so the
                          # tiles count as written (values are garbage)
                          asl = slice(0, 16 if skip_unpack else B)
                          wsl = slice(0, 16 if skip_unpack else OH)
                          if h == 0:
                              nc.vector.tensor_scalar(
                                  out=a2[4 * t + i // 2][:, i % 2, asl].bitcast(u16),
                                  in0=at_st[:, asl].bitcast(u16),
                                  scalar1=s1,
                                  scalar2=s2,
                                  op0=op0,
                                  op1=op1,
                              )
                          nc.vector.tensor_scalar(
                              out=w2[4 * t + i // 2][:, i % 2, wsl].bitcast(u16),
                              in0=wt_st[:, wsl].bitcast(u16),
                              scalar1=s1,
                              scalar2=s2,
                              op0=op0,
                              op1=op1,
                          )

                  # ---- matmul + epilogue ----
                  for b in range(NB):
                      out_st = opool.tile([P, OH], out_dt, name="out_st")
                      if psum_merge:
                          pst = ppool.tile([P, OH], f32, name="ps")
                          psums = [
                              pst[:, oq * 512 : (oq + 1) * 512]
                              for oq in range(NOQ)
                          ]
                      else:
                          psums = [
                              ppool.tile([P, 512], f32, name=f"ps_{oq}")
                              for oq in range(NOQ)
                          ]
                      for _mr in range(mm_reps):
                        for k2 in range(NK2):
                          lhsT = a2[k2][:, :, b * P : (b + 1) * P].bitcast(fp8)
                          for oq in range(NOQ):
                              nc.tensor.matmul(
                                  psums[oq],
                                  lhsT,
                                  w2[k2][:, :, oq * 512 : (oq + 1) * 512].bitcast(fp8),
                                  start=(k2 == 0),
                                  stop=(k2 == NK2 - 1),
                                  perf_mode=mybir.MatmulPerfMode.DoubleRow,
                              )
                      is_last = h == NH - 1 and b == NB - 1 and rep == reps - 1
                      epi_slices = (
                          [slice(0, OH)] if psum_merge
                          else [slice(oq * 512, (oq + 1) * 512) for oq in range(NOQ)]
                      )
                      for ei, osl in enumerate(epi_slices):
                          if skip_epilogue and not (is_last and ei == 0):
                              continue
                          nc.vector.scalar_tensor_tensor(
                              out=out_st[:, osl],
                              in0=(pst[:, osl] if psum_merge else psums[ei]),
                              scalar=r2t[:, b : b + 1],
                              in1=c_rep[:, h * OH + osl.start : h * OH + osl.stop],
                              op0=add,
                              op1=add,
                          )
                      if skip_out_dma and not is_last:
                          continue
                      # one batched output DMA per (b, phase), alternating
                      # between the two otherwise-idle issue queues
                      dma_eng = nc.scalar if b % 2 == 0 else nc.gpsimd
                      dma_eng.dma_start(
                          out=out_d[b * P : (b + 1) * P, h * OH : (h + 1) * OH],
                          in_=out_st,
                      )

    nc.compile()
    return nc


_POP = np.unpackbits(np.arange(256, dtype=np.uint8)[:, None], axis=1).sum(1)


def _prep_inputs(input_packed, weight_packed, bias, B, O, K, n_cores):
    """Host-side linear-time preprocessing: cast/transpose/shard + popcount
    rank-1 correction terms."""
    NB = B // n_cores // P
    A8 = input_packed.astype(np.uint8)  # [B, KP]
    W8 = weight_packed.astype(np.uint8)  # [O, KP]
    rA = _POP[A8].sum(1, dtype=np.int64)  # [B]
    rW = _POP[W8].sum(1, dtype=np.int64)  # [O]
    c = (bias.astype(np.float64) + K - 2.0 * rW).astype(np.float32)
    c_rep = np.ascontiguousarray(np.broadcast_to(c, (P, O)))
    r2 = (-2.0 * rA).astype(np.float32)
    at_all = np.ascontiguousarray(A8.T)  # [KP, B]
    wt = np.ascontiguousarray(W8.T)  # [KP, O]
    bsh = B // n_cores
    in_maps = []
    for ci in range(n_cores):
        sl = slice(ci * bsh, (ci + 1) * bsh)
        in_maps.append(
            {
                "at": np.ascontiguousarray(at_all[:, sl]),
                "wt": wt,
                "c_rep": c_rep,
                "r2t": np.ascontiguousarray(r2[sl].reshape(NB, P).T),
            }
        )
    return in_maps


def kernel(input_packed, weight_packed, bias):
    global LAST_RESULTS
    from concourse.bass_utils import run_bass_kernel_spmd

    input_packed = np.asarray(input_packed)
    weight_packed = np.asarray(weight_packed)
    bias = np.asarray(bias)
    B, KP = input_packed.shape
    O = weight_packed.shape[0]
    K = KP * 8
    key = (B, O, K, N_CORES)
    if key not in _NC_CACHE:
        _NC_CACHE[key] = build_program(B // N_CORES, O, K, n_devices=N_CORES)
    nc = _NC_CACHE[key]

    in_maps = _prep_inputs(input_packed, weight_packed, bias, B, O, K, N_CORES)
    res = run_bass_kernel_spmd(nc, in_maps, list(range(N_CORES)))
    LAST_RESULTS = res
    out = np.concatenate([res.results[i]["out"] for i in range(N_CORES)], axis=0)
    return np.asarray(out, dtype=np.float32)


# revision 16
# speedup vs baseline: 1.0156x; 1.0156x over previous
"""BitLinear (packed +/-1 linear layer) Trainium2 kernel.

Math: out[b,o] = sum_k a[b,k]*w[o,k] + bias[o], where a/w are +/-1 values
bit-packed LSB-first into bytes (stored as int32 0..255).

Device strategy (8 NeuronCores, data-parallel over batch):
  - Each core gets B/8 = 1024 batch rows; the full weight matrix is
    replicated. Host sends *transposed packed* uint8 tensors (k on
    partitions); the device unpacks bits into [K, *] fp8 operands.
  - Unpack: one DVE tensor_scalar per (kp-tile, bit) moves bit i of every
    byte to bit position 6 and masks: y = (x << (6-i)) & 0x40 (i=7 uses
    >> 1). Byte 0x40 bitcast to fp8e4 reads as exactly 2.0. Ops run on
    uint16 views (DVE 4x perf mode, ~355ns per [128,2048]u8 op measured).
  - fp8e4 DoubleRow matmuls (256-deep contraction), activation tile
    stationary reused across 4 PSUM banks (~120ns/matmul marginal at the
    2.4GHz max p-state; 1024 matmuls/core -> ~123us PE floor).
  - Weights stream in NH=2 phases of [K, 2048]; the w2 pool is
    double-buffered so phase h+1's DMA+unpack hides under phase h's
    matmuls (the old single-buffered pool serialized here).
  - DMA: the old kernel was DMA-bound: ~2.7us fixed cost per DMA
    instruction and a single issue queue put ~20MB of traffic at
    ~125GB/s. Now: fp16 output (8MB), batched per-kp-tile input DMAs,
    and issue spread across SP (weights), Activation (activations +
    even-b outputs), Pool (epilogue constants + odd-b outputs).
  - Identity: with a = 2*alpha-1, w = 2*omega-1 (alpha,omega in {0,1}):
      out = 4*M - 2*rowsum(alpha) - 2*rowsum(omega) + K + bias
          = psum + r2[b] + c[o]
    where r2[b] = -2*popcount_rows(A), c[o] = bias + K - 2*popcount_rows(W)
    are host precomputes. Epilogue: one scalar_tensor_tensor per psum
    bank into an fp16 staging tile, one output DMA per (b, phase).

Numerics: products in {0,4}, fp32 PSUM accumulation of integers <= 2^14 is
exact; fp32->fp16 output rounding gives |err| <= 1.0 at |out| <= ~2048
(rel ~8e-4 vs the 2e-2 gate).
"""

import os
import sys

import numpy as np

for _p in ("/opt/trn_rl_repo", "/root/.axon_site/_ro/trn_rl_repo"):
    if os.path.isdir(_p) and _p not in sys.path:
        sys.path.append(_p)

BATCH = 8192
IN_FEATURES = 4096
OUT_FEATURES = 4096
PACKED_LEN = IN_FEATURES // 8  # 512
N_CORES = 8
P = 128

_NC_CACHE: dict = {}
LAST_RESULTS = None  # stash of the most recent BassKernelResults (for test.py)


def build_program(B, O, K, n_devices=N_CORES, o_half=2048, reps=1,
                  mm_reps=1, out_bufs=3, stage_bufs=3, psum_bufs=2,
                  w2_bufs=2, out_f32=False, skip_out_dma=False,
                  skip_epilogue=False, skip_unpack=False, psum_merge=False):
    """Emit the per-core Bass/Tile program. SPMD: same program every core.

    reps>1 repeats the whole compute body (identical writes) so test.py can
    measure pure device time as (T(reps=R) - T(1)) / (R - 1). mm_reps
    repeats only the matmul block (restarting PSUM accumulation, last rep
    wins) for engine-rate microbenchmarks."""
    import concourse.bass as bass  # noqa: F401
    import concourse.mybir as mybir
    import concourse.tile as tile
    from concourse import bacc

    KP = K // 8  # packed k rows
    NT = KP // P  # kp tiles (4)
    NK2 = K // 256  # DoubleRow k-pair tiles (16)
    OH = min(O, o_half)  # o columns processed per outer phase
    NH = O // OH
    NOQ = OH // 512  # psum banks per phase
    NB = B // P  # batch tiles
    assert KP % P == 0 and O % OH == 0 and OH % 512 == 0 and B % P == 0
    assert NK2 * 2 == NT * 8 and NOQ * psum_bufs <= 8

    u8 = mybir.dt.uint8
    u16 = mybir.dt.uint16
    f32 = mybir.dt.float32
    f16 = mybir.dt.float16
    fp8 = mybir.dt.float8e4
    shl = mybir.AluOpType.logical_shift_left
    shr = mybir.AluOpType.logical_shift_right
    band = mybir.AluOpType.bitwise_and
    add = mybir.AluOpType.add
    out_dt = f32 if out_f32 else f16

    nc = bacc.Bacc(
        "TRN2",
        target_bir_lowering=False,
        debug=False,
        num_devices=n_devices,
    )

    at_d = nc.dram_tensor("at", [KP, B], u8, kind="ExternalInput").ap()
    wt_d = nc.dram_tensor("wt", [KP, O], u8, kind="ExternalInput").ap()
    c_d = nc.dram_tensor("c_rep", [P, O], f32, kind="ExternalInput").ap()
    r2_d = nc.dram_tensor("r2t", [P, NB], f32, kind="ExternalInput").ap()
    out_d = nc.dram_tensor("out", [B, O], out_dt, kind="ExternalOutput").ap()

    def unpack_ops(i):
        # Packed byte -> {0x00, 0x40} per byte lane for bit i: move the bit
        # to position 6 and mask (0x40 bitcast to fp8e4 reads as exactly
        # 2.0). Ops run on uint16 views (2 byte-lanes per element): for
        # shifts <= 6 left / 1 right, each kept bit (6 and 14) sources from
        # its own byte, so lanes stay independent under the 0x4040 mask.
        # bitVec ops keep in/out dtype equal (walrus rule) and the 16-bit
        # dtype enables the DVE 4x perf mode.
        return (shr, 1, band, 0x4040) if i == 7 else (shl, 6 - i, band, 0x4040)

    with tile.TileContext(nc) as tc:
        with (
            tc.tile_pool(name="consts", bufs=1) as cpool,
            tc.tile_pool(name="a2", bufs=1) as a2pool,
            tc.tile_pool(name="w2", bufs=w2_bufs) as w2pool,
            tc.tile_pool(name="stage", bufs=stage_bufs) as spool,
            tc.tile_pool(name="astage", bufs=2) as aspool,
            tc.tile_pool(name="outs", bufs=out_bufs) as opool,
            tc.tile_pool(name="psum", bufs=psum_bufs, space="PSUM") as ppool,
        ):
            c_rep = cpool.tile([P, O], f32, name="c_rep_t")
            r2t = cpool.tile([P, NB], f32, name="r2t_t")
            consts_loaded = False

            def prep_chunk(w2_tiles, a2_tiles, h, t):
                """DMA + unpack one kp-tile's worth of a phase's operands."""
                nonlocal consts_loaded
                wt_st = spool.tile([P, OH], u8, name="wt_st")
                nc.sync.dma_start(
                    out=wt_st,
                    in_=wt_d[t * P : (t + 1) * P, h * OH : (h + 1) * OH],
                )
                if a2_tiles is not None:
                    at_st = aspool.tile([P, B], u8, name="at_st")
                    nc.scalar.dma_start(
                        out=at_st, in_=at_d[t * P : (t + 1) * P, :]
                    )
                if not consts_loaded:
                    # on the idle Pool queue: lands well before the first
                    # epilogue without delaying critical tiles
                    consts_loaded = True
                    nc.gpsimd.dma_start(out=r2t, in_=r2_d)
                    nc.gpsimd.dma_start(out=c_rep, in_=c_d)
                for i in range(8):
                    op0, s1, op1, s2 = unpack_ops(i)
                    # skip_unpack: timing-only variant — replace the
                    # full-width unpack with a 16-element write so the
                    # tiles count as written (values are garbage)
                    asl = slice(0, 16 if skip_unpack else B)
                    wsl = slice(0, 16 if skip_unpack else OH)
                    if a2_tiles is not None:
                        nc.vector.tensor_scalar(
                            out=a2_tiles[4 * t + i // 2][:, i % 2, asl].bitcast(u16),
                            in0=at_st[:, asl].bitcast(u16),
                            scalar1=s1,
                            scalar2=s2,
                            op0=op0,
                            op1=op1,
                        )
                    nc.vector.tensor_scalar(
                        out=w2_tiles[4 * t + i // 2][:, i % 2, wsl].bitcast(u16),
                        in0=wt_st[:, wsl].bitcast(u16),
                        scalar1=s1,
                        scalar2=s2,
                        op0=op0,
                        op1=op1,
                    )

            def new_w2():
                return [
                    w2pool.tile([P, 2, OH], u8, name=f"w2_{k2}")
                    for k2 in range(NK2)
                ]

            # repeat body for delta timing (rep>0 re-does identical work).
            # Phase-pipelined emission: the NEXT phase's weight DMA+unpack
            # chunks are emitted spread between the current phase's
            # b-iterations, so the in-order DVE queue interleaves unpack ops
            # with epilogue bursts instead of serializing a whole phase's
            # unpack behind them (w2 pool double-buffered; deps are
            # tile-tracked, emission order is a scheduling hint). The
            # activation unpack is single-buffered and WAR-blocked on the
            # previous rep's last reader, so it stays at rep start.
            w2_next = None
            for rep in range(reps):
              a2 = [
                  a2pool.tile([P, 2, B], u8, name=f"a2_{k2}")
                  for k2 in range(NK2)
              ]
              if w2_next is not None:
                  # this rep's h=0 weights were prefetched during the
                  # previous rep; emit the activation DMA+unpack standalone
                  for t in range(NT):
                      at_st = aspool.tile([P, B], u8, name="at_st")
                      nc.scalar.dma_start(
                          out=at_st, in_=at_d[t * P : (t + 1) * P, :]
                      )
                      for i in range(8):
                          op0, s1, op1, s2 = unpack_ops(i)
                          asl = slice(0, 16 if skip_unpack else B)
                          nc.vector.tensor_scalar(
                              out=a2[4 * t + i // 2][:, i % 2, asl].bitcast(u16),
                              in0=at_st[:, asl].bitcast(u16),
                              scalar1=s1,
                              scalar2=s2,
                              op0=op0,
                              op1=op1,
                          )
              for h in range(NH):
                  if w2_next is None:
                      # nothing prefetched (first phase): emit this phase's
                      # prep up-front, activation unpack interleaved k2-major
                      # so the PE can start on the first a2/w2 pair
                      w2 = new_w2()
                      for t in range(NT):
                          prep_chunk(w2, a2 if h == 0 else None, h, t)
                  else:
                      w2 = w2_next
                  # decide what to prefetch during this phase's b-loop
                  if h + 1 < NH:
                      nxt = h + 1
                  elif rep + 1 < reps:
                      nxt = 0  # next rep's h=0 weights (acts at rep start)
                  else:
                      nxt = None
                  w2_next = new_w2() if nxt is not None else None

                  # ---- matmul + epilogue ----
                  for b in range(NB):
                      out_st = opool.tile([P, OH], out_dt, name="out_st")
                      if psum_merge:
                          pst = ppool.tile([P, OH], f32, name="ps")
                          psums = [
                              pst[:, oq * 512 : (oq + 1) * 512]
                              for oq in range(NOQ)
                          ]
                      else:
                          psums = [
                              ppool.tile([P, 512], f32, name=f"ps_{oq}")
                              for oq in range(NOQ)
                          ]
                      for _mr in range(mm_reps):
                        for k2 in range(NK2):
                          lhsT = a2[k2][:, :, b * P : (b + 1) * P].bitcast(fp8)
                          for oq in range(NOQ):
                              nc.tensor.matmul(
                                  psums[oq],
                                  lhsT,
                                  w2[k2][:, :, oq * 512 : (oq + 1) * 512].bitcast(fp8),
                                  start=(k2 == 0),
                                  stop=(k2 == NK2 - 1),
                                  perf_mode=mybir.MatmulPerfMode.DoubleRow,
                              )
                      # interleave one prep chunk of the next phase per
                      # other b-iteration (4 chunks over 8 b's)
                      if w2_next is not None and b % 2 == 0:
                          prep_chunk(w2_next, None, nxt, b // 2)
                      is_last = h == NH - 1 and b == NB - 1 and rep == reps - 1
                      epi_slices = (
                          [slice(0, OH)] if psum_merge
                          else [slice(oq * 512, (oq + 1) * 512) for oq in range(NOQ)]
                      )
                      for ei, osl in enumerate(epi_slices):
                          if skip_epilogue and not (is_last and ei == 0):
                              continue
                          nc.vector.scalar_tensor_tensor(
                              out=out_st[:, osl],
                              in0=(pst[:, osl] if psum_merge else psums[ei]),
                              scalar=r2t[:, b : b + 1],
                              in1=c_rep[:, h * OH + osl.start : h * OH + osl.stop],
                              op0=add,
                              op1=add,
                          )
                      if skip_out_dma and not is_last:
                          continue
                      # one batched output DMA per (b, phase), alternating
                      # between the two otherwise-idle issue queues
                      dma_eng = nc.scalar if b % 2 == 0 else nc.gpsimd
                      dma_eng.dma_start(
                          out=out_d[b * P : (b + 1) * P, h * OH : (h + 1) * OH],
                          in_=out_st,
                      )

    nc.compile()
    return nc


_POP = np.unpackbits(np.arange(256, dtype=np.uint8)[:, None], axis=1).sum(1)


def _prep_inputs(input_packed, weight_packed, bias, B, O, K, n_cores):
    """Host-side linear-time preprocessing: cast/transpose/shard + popcount
    rank-1 correction terms."""
    NB = B // n_cores // P
    A8 = input_packed.astype(np.uint8)  # [B, KP]
    W8 = weight_packed.astype(np.uint8)  # [O, KP]
    rA = _POP[A8].sum(1, dtype=np.int64)  # [B]
    rW = _POP[W8].sum(1, dtype=np.int64)  # [O]
    c = (bias.astype(np.float64) + K - 2.0 * rW).astype(np.float32)
    c_rep = np.ascontiguousarray(np.broadcast_to(c, (P, O)))
    r2 = (-2.0 * rA).astype(np.float32)
    at_all = np.ascontiguousarray(A8.T)  # [KP, B]
    wt = np.ascontiguousarray(W8.T)  # [KP, O]
    bsh = B // n_cores
    in_maps = []
    for ci in range(n_cores):
        sl = slice(ci * bsh, (ci + 1) * bsh)
        in_maps.append(
            {
                "at": np.ascontiguousarray(at_all[:, sl]),
                "wt": wt,
                "c_rep": c_rep,
                "r2t": np.ascontiguousarray(r2[sl].reshape(NB, P).T),
            }
        )
    return in_maps


def kernel(input_packed, weight_packed, bias):
    global LAST_RESULTS
    from concourse.bass_utils import run_bass_kernel_spmd

    input_packed = np.asarray(input_packed)
    weight_packed = np.asarray(weight_packed)
    bias = np.asarray(bias)
    B, KP = input_packed.shape
    O = weight_packed.shape[0]
    K = KP * 8
    key = (B, O, K, N_CORES)
    if key not in _NC_CACHE:
        _NC_CACHE[key] = build_program(B // N_CORES, O, K, n_devices=N_CORES)
    nc = _NC_CACHE[key]

    in_maps = _prep_inputs(input_packed, weight_packed, bias, B, O, K, N_CORES)
    res = run_bass_kernel_spmd(nc, in_maps, list(range(N_CORES)))
    LAST_RESULTS = res
    out = np.concatenate([res.results[i]["out"] for i in range(N_CORES)], axis=0)
    return np.asarray(out, dtype=np.float32)


# revision 21
# speedup vs baseline: 1.0598x; 1.0435x over previous
"""BitLinear (packed +/-1 linear layer) Trainium2 kernel.

Math: out[b,o] = sum_k a[b,k]*w[o,k] + bias[o], where a/w are +/-1 values
bit-packed LSB-first into bytes (stored as int32 0..255).

Device strategy (8 NeuronCores, data-parallel over batch):
  - Each core gets B/8 = 1024 batch rows; the full weight matrix is
    replicated. Host sends *transposed packed* uint8 tensors (k on
    partitions); the device unpacks bits into [K, *] fp8 operands.
  - Unpack: one DVE tensor_scalar per (kp-tile, bit) moves bit i of every
    byte to bit position 6 and masks: y = (x << (6-i)) & 0x40 (i=7 uses
    >> 1). Byte 0x40 bitcast to fp8e4 reads as exactly 2.0. Ops run on
    uint16 views (DVE 4x perf mode, ~355ns per [128,2048]u8 op measured).
  - fp8e4 DoubleRow matmuls (256-deep contraction), activation tile
    stationary reused across 4 PSUM banks. Measured ~145ns/matmul on HW
    including the per-matmul InstLdweights (1024 matmuls/core -> ~149us
    PE floor; an ablation with epilogue/unpack/output removed hits the
    same number, so the non-PE work is fully hidden).
  - Weights stream in NH=2 phases of [K, 2048]; the w2 pool is
    double-buffered so phase h+1's DMA+unpack hides under phase h's
    matmuls (the old single-buffered pool serialized here).
  - DMA: the old kernel was DMA-limited: ~2.7us fixed cost per DMA
    instruction and a single issue queue carried ~20MB of traffic
    (64 separate per-bank output stores). Now: fp16 output (8MB, exact
    here: outputs are integers <= 2048 plus ~0.01-scale bias, and such
    integers are fp16-representable), one batched output DMA per
    (b, phase), and issue spread across SP (weights), Activation
    (activations + even-b outputs), Pool (epilogue constants + odd-b
    outputs).
  - Identity: with a = 2*alpha-1, w = 2*omega-1 (alpha,omega in {0,1}):
      out = 4*M - 2*rowsum(alpha) - 2*rowsum(omega) + K + bias
          = psum + r2[b] + c[o]
    where r2[b] = -2*popcount_rows(A), c[o] = bias + K - 2*popcount_rows(W)
    are host precomputes. Epilogue: one scalar_tensor_tensor per psum
    bank into an fp16 staging tile, one output DMA per (b, phase).

Numerics: products in {0,4}, fp32 PSUM accumulation of integers <= 2^14 is
exact; fp32->fp16 output rounding gives |err| <= 1.0 at |out| <= ~2048
(rel ~8e-4 vs the 2e-2 gate).
"""

import os
import sys

import numpy as np

for _p in ("/opt/trn_rl_repo", "/root/.axon_site/_ro/trn_rl_repo"):
    if os.path.isdir(_p) and _p not in sys.path:
        sys.path.append(_p)

BATCH = 8192
IN_FEATURES = 4096
OUT_FEATURES = 4096
PACKED_LEN = IN_FEATURES // 8  # 512
N_CORES = 8
P = 128

_NC_CACHE: dict = {}
LAST_RESULTS = None  # stash of the most recent BassKernelResults (for test.py)


def build_program(B, O, K, n_devices=N_CORES, o_half=2048, reps=1,
                  mm_reps=1, out_bufs=3, stage_bufs=3, psum_bufs=2,
                  w2_bufs=2, out_f32=False, skip_out_dma=False,
                  skip_epilogue=False, skip_unpack=False, psum_merge=False,
                  interleave=False):
    """Emit the per-core Bass/Tile program. SPMD: same program every core.

    reps>1 repeats the whole compute body (identical writes) so test.py can
    measure pure device time as (T(reps=R) - T(1)) / (R - 1). mm_reps
    repeats only the matmul block (restarting PSUM accumulation, last rep
    wins) for engine-rate microbenchmarks."""
    import concourse.bass as bass  # noqa: F401
    import concourse.mybir as mybir
    import concourse.tile as tile
    from concourse import bacc

    KP = K // 8  # packed k rows
    NT = KP // P  # kp tiles (4)
    NK2 = K // 256  # DoubleRow k-pair tiles (16)
    OH = min(O, o_half)  # o columns processed per outer phase
    NH = O // OH
    NOQ = OH // 512  # psum banks per phase
    NB = B // P  # batch tiles
    assert KP % P == 0 and O % OH == 0 and OH % 512 == 0 and B % P == 0
    assert NK2 * 2 == NT * 8 and NOQ * psum_bufs <= 8

    u8 = mybir.dt.uint8
    u16 = mybir.dt.uint16
    f32 = mybir.dt.float32
    f16 = mybir.dt.float16
    fp8 = mybir.dt.float8e4
    shl = mybir.AluOpType.logical_shift_left
    shr = mybir.AluOpType.logical_shift_right
    band = mybir.AluOpType.bitwise_and
    add = mybir.AluOpType.add
    out_dt = f32 if out_f32 else f16

    nc = bacc.Bacc(
        "TRN2",
        target_bir_lowering=False,
        debug=False,
        num_devices=n_devices,
    )

    at_d = nc.dram_tensor("at", [KP, B], u8, kind="ExternalInput").ap()
    wt_d = nc.dram_tensor("wt", [KP, O], u8, kind="ExternalInput").ap()
    c_d = nc.dram_tensor("c_rep", [P, O], f32, kind="ExternalInput").ap()
    r2_d = nc.dram_tensor("r2t", [P, NB], f32, kind="ExternalInput").ap()
    out_d = nc.dram_tensor("out", [B, O], out_dt, kind="ExternalOutput").ap()

    def unpack_ops(i):
        # Packed byte -> {0x00, 0x40} per byte lane for bit i: move the bit
        # to position 6 and mask (0x40 bitcast to fp8e4 reads as exactly
        # 2.0). Ops run on uint16 views (2 byte-lanes per element): for
        # shifts <= 6 left / 1 right, each kept bit (6 and 14) sources from
        # its own byte, so lanes stay independent under the 0x4040 mask.
        # bitVec ops keep in/out dtype equal (walrus rule) and the 16-bit
        # dtype enables the DVE 4x perf mode.
        return (shr, 1, band, 0x4040) if i == 7 else (shl, 6 - i, band, 0x4040)

    with tile.TileContext(nc) as tc:
        with (
            tc.tile_pool(name="consts", bufs=1) as cpool,
            tc.tile_pool(name="a2", bufs=1) as a2pool,
            tc.tile_pool(name="w2", bufs=w2_bufs) as w2pool,
            tc.tile_pool(name="stage", bufs=stage_bufs) as spool,
            tc.tile_pool(name="astage", bufs=2) as aspool,
            tc.tile_pool(name="outs", bufs=out_bufs) as opool,
            tc.tile_pool(name="psum", bufs=psum_bufs, space="PSUM") as ppool,
        ):
            c_rep = cpool.tile([P, O], f32, name="c_rep_t")
            r2t = cpool.tile([P, NB], f32, name="r2t_t")
            consts_loaded = False

            def prep_chunk(w2_tiles, a2_tiles, h, t):
                """DMA + unpack one kp-tile's worth of a phase's operands."""
                nonlocal consts_loaded
                wt_st = spool.tile([P, OH], u8, name="wt_st")
                nc.sync.dma_start(
                    out=wt_st,
                    in_=wt_d[t * P : (t + 1) * P, h * OH : (h + 1) * OH],
                )
                if a2_tiles is not None:
                    at_st = aspool.tile([P, B], u8, name="at_st")
                    nc.scalar.dma_start(
                        out=at_st, in_=at_d[t * P : (t + 1) * P, :]
                    )
                if not consts_loaded:
                    # on the idle Pool queue: lands well before the first
                    # epilogue without delaying critical tiles
                    consts_loaded = True
                    nc.gpsimd.dma_start(out=r2t, in_=r2_d)
                    nc.gpsimd.dma_start(out=c_rep, in_=c_d)
                for i in range(8):
                    op0, s1, op1, s2 = unpack_ops(i)
                    # skip_unpack: timing-only variant — replace the
                    # full-width unpack with a 16-element write so the
                    # tiles count as written (values are garbage)
                    asl = slice(0, 16 if skip_unpack else B)
                    wsl = slice(0, 16 if skip_unpack else OH)
                    if a2_tiles is not None:
                        nc.vector.tensor_scalar(
                            out=a2_tiles[4 * t + i // 2][:, i % 2, asl].bitcast(u16),
                            in0=at_st[:, asl].bitcast(u16),
                            scalar1=s1,
                            scalar2=s2,
                            op0=op0,
                            op1=op1,
                        )
                    nc.vector.tensor_scalar(
                        out=w2_tiles[4 * t + i // 2][:, i % 2, wsl].bitcast(u16),
                        in0=wt_st[:, wsl].bitcast(u16),
                        scalar1=s1,
                        scalar2=s2,
                        op0=op0,
                        op1=op1,
                    )

            def new_w2():
                return [
                    w2pool.tile([P, 2, OH], u8, name=f"w2_{k2}")
                    for k2 in range(NK2)
                ]

            # repeat body for delta timing (rep>0 re-does identical work).
            # Phase-pipelined emission: the NEXT phase's weight DMA+unpack
            # chunks are emitted spread between the current phase's
            # b-iterations, so the in-order DVE queue interleaves unpack ops
            # with epilogue bursts instead of serializing a whole phase's
            # unpack behind them (w2 pool double-buffered; deps are
            # tile-tracked, emission order is a scheduling hint). The
            # activation unpack is single-buffered and WAR-blocked on the
            # previous rep's last reader, so it stays at rep start.
            w2_next = None
            for rep in range(reps):
              a2 = [
                  a2pool.tile([P, 2, B], u8, name=f"a2_{k2}")
                  for k2 in range(NK2)
              ]
              if w2_next is not None:
                  # this rep's h=0 weights were prefetched during the
                  # previous rep; emit the activation DMA+unpack standalone
                  for t in range(NT):
                      at_st = aspool.tile([P, B], u8, name="at_st")
                      nc.scalar.dma_start(
                          out=at_st, in_=at_d[t * P : (t + 1) * P, :]
                      )
                      for i in range(8):
                          op0, s1, op1, s2 = unpack_ops(i)
                          asl = slice(0, 16 if skip_unpack else B)
                          nc.vector.tensor_scalar(
                              out=a2[4 * t + i // 2][:, i % 2, asl].bitcast(u16),
                              in0=at_st[:, asl].bitcast(u16),
                              scalar1=s1,
                              scalar2=s2,
                              op0=op0,
                              op1=op1,
                          )
              for h in range(NH):
                  if w2_next is None:
                      # nothing prefetched (first phase): emit this phase's
                      # prep up-front, activation unpack interleaved k2-major
                      # so the PE can start on the first a2/w2 pair
                      w2 = new_w2()
                      for t in range(NT):
                          prep_chunk(w2, a2 if h == 0 else None, h, t)
                  else:
                      w2 = w2_next
                  # decide what to prefetch during this phase's b-loop
                  if not interleave:
                      nxt = None  # v3-style: every phase preps up-front
                  elif h + 1 < NH:
                      nxt = h + 1
                  elif rep + 1 < reps:
                      nxt = 0  # next rep's h=0 weights (acts at rep start)
                  else:
                      nxt = None
                  w2_next = new_w2() if nxt is not None else None

                  # ---- matmul + epilogue ----
                  for b in range(NB):
                      out_st = opool.tile([P, OH], out_dt, name="out_st")
                      if psum_merge:
                          pst = ppool.tile([P, OH], f32, name="ps")
                          psums = [
                              pst[:, oq * 512 : (oq + 1) * 512]
                              for oq in range(NOQ)
                          ]
                      else:
                          psums = [
                              ppool.tile([P, 512], f32, name=f"ps_{oq}")
                              for oq in range(NOQ)
                          ]
                      for _mr in range(mm_reps):
                        for k2 in range(NK2):
                          lhsT = a2[k2][:, :, b * P : (b + 1) * P].bitcast(fp8)
                          for oq in range(NOQ):
                              nc.tensor.matmul(
                                  psums[oq],
                                  lhsT,
                                  w2[k2][:, :, oq * 512 : (oq + 1) * 512].bitcast(fp8),
                                  start=(k2 == 0),
                                  stop=(k2 == NK2 - 1),
                                  perf_mode=mybir.MatmulPerfMode.DoubleRow,
                              )
                      # interleave one prep chunk of the next phase per
                      # other b-iteration (4 chunks over 8 b's)
                      if w2_next is not None and b % 2 == 0:
                          prep_chunk(w2_next, None, nxt, b // 2)
                      is_last = h == NH - 1 and b == NB - 1 and rep == reps - 1
                      epi_slices = (
                          [slice(0, OH)] if psum_merge
                          else [slice(oq * 512, (oq + 1) * 512) for oq in range(NOQ)]
                      )
                      for ei, osl in enumerate(epi_slices):
                          if skip_epilogue and not (is_last and ei == 0):
                              continue
                          nc.vector.scalar_tensor_tensor(
                              out=out_st[:, osl],
                              in0=(pst[:, osl] if psum_merge else psums[ei]),
                              scalar=r2t[:, b : b + 1],
                              in1=c_rep[:, h * OH + osl.start : h * OH + osl.stop],
                              op0=add,
                              op1=add,
                          )
                      if skip_out_dma and not is_last:
                          continue
                      # one batched output DMA per (b, phase), alternating
                      # between the two otherwise-idle issue queues
                      dma_eng = nc.scalar if b % 2 == 0 else nc.gpsimd
                      dma_eng.dma_start(
                          out=out_d[b * P : (b + 1) * P, h * OH : (h + 1) * OH],
                          in_=out_st,
                      )

    nc.compile()
    return nc


_POP = np.unpackbits(np.arange(256, dtype=np.uint8)[:, None], axis=1).sum(1)


def _prep_inputs(input_packed, weight_packed, bias, B, O, K, n_cores):
    """Host-side linear-time preprocessing: cast/transpose/shard + popcount
    rank-1 correction terms."""
    NB = B // n_cores // P
    A8 = input_packed.astype(np.uint8)  # [B, KP]
    W8 = weight_packed.astype(np.uint8)  # [O, KP]
    rA = _POP[A8].sum(1, dtype=np.int64)  # [B]
    rW = _POP[W8].sum(1, dtype=np.int64)  # [O]
    c = (bias.astype(np.float64) + K - 2.0 * rW).astype(np.float32)
    c_rep = np.ascontiguousarray(np.broadcast_to(c, (P, O)))
    r2 = (-2.0 * rA).astype(np.float32)
    at_all = np.ascontiguousarray(A8.T)  # [KP, B]
    wt = np.ascontiguousarray(W8.T)  # [KP, O]
    bsh = B // n_cores
    in_maps = []
    for ci in range(n_cores):
        sl = slice(ci * bsh, (ci + 1) * bsh)
        in_maps.append(
            {
                "at": np.ascontiguousarray(at_all[:, sl]),
                "wt": wt,
                "c_rep": c_rep,
                "r2t": np.ascontiguousarray(r2[sl].reshape(NB, P).T),
            }
        )
    return in_maps


def kernel(input_packed, weight_packed, bias):
    global LAST_RESULTS
    from concourse.bass_utils import run_bass_kernel_spmd

    input_packed = np.asarray(input_packed)
    weight_packed = np.asarray(weight_packed)
    bias = np.asarray(bias)
    B, KP = input_packed.shape
    O = weight_packed.shape[0]
    K = KP * 8
    key = (B, O, K, N_CORES)
    if key not in _NC_CACHE:
        _NC_CACHE[key] = build_program(B // N_CORES, O, K, n_devices=N_CORES)
    nc = _NC_CACHE[key]

    in_maps = _prep_inputs(input_packed, weight_packed, bias, B, O, K, N_CORES)
    res = run_bass_kernel_spmd(nc, in_maps, list(range(N_CORES)))
    LAST_RESULTS = res
    out = np.concatenate([res.results[i]["out"] for i in range(N_CORES)], axis=0)
    return np.asarray(out, dtype=np.float32)
